# revision 16
# baseline (speedup 1.0000x reference)
"""SimCLR-style contrastive loss (nn_Contrast) on 8 Trainium2 NeuronCores.

Strategy (data-parallel over rows of z = concat(normalize(x_i), normalize(x_j))):
  - Host rotates z's raw rows by -c*1024 per core, so each core's slab is
    always rotated rows [0, 1024) and the positive-pair partners are always
    rotated rows [4096, 5120): one SPMD program, no per-core divergence.
  - On device: normalize all 8192 rows (fp32 norms, bf16 output), DMA-xbar
    transpose to zT [256, 8192] bf16, then compute the slab's [1024, 8192]
    similarity block with bf16 matmuls accumulated in PSUM fp32, apply
    exp(2*sim) on ScalarE in-place in PSUM with accum_out producing the
    row-sums S_i directly.  pos_i comes from an elementwise dot of raw rows
    with their partner rows (fp32).
  - Device returns S [128, 8] and pos2 = sim(i, i+B)/TEMP [128, 8] per core;
    host computes loss = -pos2 + log(S - e^2 + exp(pos2)) and the mean.
    (diag(sim) == 1/TEMP exactly for normalized rows, so exp(diag) = e^2.)
"""

import numpy as np

B = 4096
D = 256
NB = 2 * B            # 8192 rows of z
NCORES = 8
SLAB = NB // NCORES   # 1024 rows per core
NT = NB // 128        # 64 row tiles
IT = SLAB // 128      # 8 slab row tiles
TEMP = 0.5
INV_TEMP = 1.0 / TEMP
E2 = float(np.exp(INV_TEMP))

GRP = 8               # row tiles per rinorm batch
ACT_J = 1024          # j-span per big exp instruction (2 PSUM banks)

_nc_cache = None


def _patch_tile_drain():
    """This container's walrus accepts at most ONE sem-wait per instruction,
    but Tile's wait assignment can attach several (and the tail drain gets
    one per busy proc).  Legalize by hoisting extra waits onto preceding
    same-engine NoOps (same semantics: an engine executes its stream in
    order, and multi-waits are AND conditions)."""
    import concourse.tile as tile
    from concourse import mybir
    from concourse.vector_clock import ScopedClock

    if getattr(tile.TileContext, "_drain_patch_applied", False):
        return

    _ctr = [0]

    def _legalize_waits(nc):
        for f in nc.m.functions:
            for bb in f.blocks:
                insts = bb.instructions
                new = []
                changed = False
                for inst in insts:
                    si = inst.sync_info
                    waits = list(si.on_wait) if (si and si.on_wait) else []
                    if len(waits) > 1:
                        for w in waits[:-1]:
                            _ctr[0] += 1
                            nop = mybir.InstNoOp(
                                name=f"legalize-wait-{_ctr[0]}", ins=[], outs=[]
                            )
                            nop.engine = inst.engine
                            nop.sync_info = mybir.SyncInfo(
                                on_wait=[w], on_update=[]
                            )
                            new.append(nop)
                        si.on_wait = [waits[-1]]
                        changed = True
                    new.append(inst)
                if changed:
                    bb.instructions = new

    def _drain_and_barrier(self, tick_clock, wait_clock):
        nc = self.nc
        nop0 = nc.sync.nop()
        wait_clock.add_sem_waits(
            nop0.ins, ScopedClock({None: tick_clock.global_clock})
        )
        nc.sync.drain()
        nc.all_engine_barrier()
        assert self.sems is not None
        popped = nc._tile_sem_poison_stack.pop()
        assert popped is self._sem_poison
        nc.clear_and_free_semaphores(list(self.sems.allocated().values()))
        nc.all_engine_barrier()
        _legalize_waits(nc)

    tile.TileContext._drain_and_barrier = _drain_and_barrier
    tile.TileContext._drain_patch_applied = True


def _build_nc(repeat=1, parts='full'):
    from concourse import mybir, masks
    import concourse.bass as bass
    import concourse.tile as tile
    import contextlib

    _patch_tile_drain()

    f32 = mybir.dt.float32
    bf16 = mybir.dt.bfloat16
    Act = mybir.ActivationFunctionType
    Alu = mybir.AluOpType

    nc = bass.Bass()
    z_dram = nc.dram_tensor("z", [NB, D], f32, kind="ExternalInput")
    s_dram = nc.dram_tensor("s_out", [128, IT], f32, kind="ExternalOutput")
    p2_dram = nc.dram_tensor("p2_out", [128, IT], f32, kind="ExternalOutput")

    with tile.TileContext(nc) as tc:
        rep_ctx = tc.For_i(0, repeat) if repeat > 1 else contextlib.nullcontext()
        with (
            rep_ctx,
            tc.tile_pool(name="persist", bufs=1) as persist,
            tc.tile_pool(name="scratch", bufs=4) as scratch,
            tc.tile_pool(name="psum", bufs=3, space="PSUM") as psum,
            tc.tile_pool(name="psum_tp", bufs=2, space="PSUM") as psum_tp,
        ):
            zraw = persist.tile([128, NT, D], f32, tag="zraw")
            zbf = persist.tile([128, NT, D], bf16, tag="zbf")
            zT = [
                persist.tile([128, NB], bf16, tag="zT0", name="zT0"),
                persist.tile([128, NB], bf16, tag="zT1", name="zT1"),
            ]
            norms2 = persist.tile([128, NT], f32, tag="norms2")
            lnb = persist.tile([128, NT], f32, tag="lnb")
            rinorm = persist.tile([128, NT], f32, tag="rinorm")
            accum = persist.tile([128, IT * (NB // ACT_J)], f32, tag="accum")
            s_tile = persist.tile([128, IT], f32, tag="s_tile")
            dotraw = persist.tile([128, IT], f32, tag="dotraw")
            tmp8 = persist.tile([128, IT], f32, tag="tmp8")
            pos2 = persist.tile([128, IT], f32, tag="pos2")
            ident = persist.tile([128, 128], bf16, tag="ident")
            masks.make_identity(nc, ident[:])

            # ---- load + normalize pipeline, in groups of GRP row tiles ----
            for g in range(NT // GRP):
                t0 = g * GRP
                for t in range(t0, t0 + GRP):
                    nc.sync.dma_start(
                        out=zraw[:, t, :], in_=z_dram[t * 128 : (t + 1) * 128, :]
                    )
                    sq = scratch.tile([128, D], f32, tag="sq_scratch")
                    nc.vector.scalar_tensor_tensor(
                        out=sq,
                        in0=zraw[:, t, :],
                        scalar=1.0,
                        in1=zraw[:, t, :],
                        op0=Alu.mult,
                        op1=Alu.mult,
                        accum_out=norms2[:, t : t + 1],
                    )
                # rinorm = exp(-0.5 * ln(sumsq)) : one table set (ln+exp)
                gs = slice(t0, t0 + GRP)
                nc.scalar.activation(
                    out=lnb[:, gs], in_=norms2[:, gs], func=Act.Ln
                )
                nc.scalar.activation(
                    out=rinorm[:, gs], in_=lnb[:, gs], func=Act.Exp, scale=-0.5
                )
                for t in range(t0, t0 + GRP):
                    nc.vector.tensor_scalar_mul(
                        zbf[:, t, :], zraw[:, t, :], rinorm[:, t : t + 1]
                    )
                # transpose via PE (idle in this phase): 4 tiles x 2 halves
                # per PSUM buffer, then one Pool copy per half into zT.
                for half in range(2):
                    tp = psum_tp.tile([128, 1024], bf16, tag="tp")
                    tb = t0 + half * 4
                    for d in range(2):
                        for k in range(4):
                            nc.tensor.transpose(
                                tp[:, (d * 4 + k) * 128 : (d * 4 + k + 1) * 128],
                                zbf[:, tb + k, d * 128 : (d + 1) * 128],
                                ident,
                            )
                    for d in range(2):
                        nc.scalar.copy(
                            zT[d][:, tb * 128 : tb * 128 + 512],
                            tp[:, d * 512 : (d + 1) * 512],
                        )

            # ---- positive pairs: raw dot of slab rows with partner rows ----
            for t in range(IT):
                pscr = scratch.tile([128, D], f32, tag="sq_scratch")
                nc.vector.scalar_tensor_tensor(
                    out=pscr,
                    in0=zraw[:, t, :],
                    scalar=1.0,
                    in1=zraw[:, t + 32, :],
                    op0=Alu.mult,
                    op1=Alu.mult,
                    accum_out=dotraw[:, t : t + 1],
                )
            # pos2 = (dotraw * INV_TEMP) * (rinorm_slab * rinorm_partner)
            nc.vector.tensor_mul(tmp8, rinorm[:, 0:IT], rinorm[:, 32 : 32 + IT])
            nc.vector.scalar_tensor_tensor(
                out=pos2,
                in0=dotraw,
                scalar=float(INV_TEMP),
                in1=tmp8,
                op0=Alu.mult,
                op1=Alu.mult,
            )
            nc.sync.dma_start(out=p2_dram[:, :], in_=pos2)

            # ---- main loop: sim slab x exp + row-sum accumulation ----
            nq = NB // ACT_J            # 4 psum tiles per slab i-tile
            jc_per_q = ACT_J // 512     # 4 matmuls of 512 per d step
            for it in range(IT if parts in ('full', 'main') else 0):
                for q in range(nq):
                    pt = psum.tile([128, ACT_J], f32, tag="pt")
                    for d in range(2):
                        for jc in range(jc_per_q):
                            j0 = q * ACT_J + jc * 512
                            nc.tensor.matmul(
                                pt[:, jc * 512 : (jc + 1) * 512],
                                lhsT=zT[d][:, it * 128 : (it + 1) * 128],
                                rhs=zT[d][:, j0 : j0 + 512],
                                start=(d == 0),
                                stop=(d == 1),
                            )
                    col = it * nq + q
                    nc.scalar.activation(
                        out=pt,
                        in_=pt,
                        func=Act.Exp,
                        scale=float(INV_TEMP),
                        accum_out=accum[:, col : col + 1],
                    )

            if parts in ('load',):
                nc.vector.memset(accum, 1.0)
            # S[:, it] = sum of the nq partial row-sums
            nc.vector.tensor_reduce(
                out=s_tile,
                in_=accum.rearrange("p (a b) -> p a b", b=nq),
                axis=mybir.AxisListType.X,
                op=Alu.add,
            )
            nc.sync.dma_start(out=s_dram[:, :], in_=s_tile)

    return nc


def _get_nc():
    global _nc_cache
    if _nc_cache is None:
        _nc_cache = _build_nc()
    return _nc_cache


def kernel(x_i, x_j):
    from concourse import bass_utils

    z = np.concatenate(
        [np.asarray(x_i, dtype=np.float32), np.asarray(x_j, dtype=np.float32)], axis=0
    )
    in_maps = [
        {"z": np.ascontiguousarray(np.roll(z, -c * SLAB, axis=0))}
        for c in range(NCORES)
    ]
    nc = _get_nc()
    res = bass_utils.run_bass_kernel_spmd(nc, in_maps, core_ids=list(range(NCORES)))

    S = np.stack([res.results[c]["s_out"] for c in range(NCORES)]).astype(np.float64)
    P2 = np.stack([res.results[c]["p2_out"] for c in range(NCORES)]).astype(np.float64)
    loss = -P2 + np.log(S - E2 + np.exp(P2))
    return np.array(loss.mean(), dtype=np.float32)
